# revision 1
# baseline (speedup 1.0000x reference)
"""Balanced dice loss (histogram binning) on 8 Trainium2 NeuronCores.

Math: with t ∈ {0,1} and p = sigmoid(x), the loss needs four global sums:
    S_t   = Σ t            (the bincount)
    S_pt  = Σ p·t
    S_pp  = Σ p²
    S_ppt = Σ p²·t
Then with c1 = S_t, c0 = N − c1, w0 = 1/(c0+s)², w1 = 1/(c1+s)²:
    intersection = w1·S_pt
    denominator  = w0·(S_pp − S_ppt) + w1·(S_ppt + c1)
    dice = 1 − (2·I + s)/(D + s)

Device kernel (data-parallel over 8 cores, batch-sharded), per [128,F] tile:
    ACT : p = sigmoid(x) (bf16); tb = copy(t) int32→bf16 with row-accum
          → S_t (one pass does the dtype conversion AND the bincount)
    DVE : u = p·tb, w = u·p (= p²·t), sq = p·p — all-bf16 2× perf mode
    PE  : ones[128,128] @ 512-col chunks of u, w, sq → three PSUM
          column-sum accumulation chains (S_pt, S_ppt, S_pp), each
          alternating two banks to pipeline the RMW
p/tb and u/w/sq live in two merged pool tiles (fewer pool allocations →
shorter end-of-kernel semaphore teardown). The last tile computes
u = p·t from the int32 tile directly (1×) so u doesn't wait on the S_t
copy, and runs its DVE work in 512-col sub-slices so the PE chains
drain as chunks are produced. Partials are DMA'd out; host reduces in
float64.
"""

import numpy as np

import concourse.bacc as bacc
import concourse.mybir as mybir
from concourse.bass_utils import run_bass_kernel_spmd
from concourse.tile import TileContext

N_CORES = 8
P = 128
TOTAL = 32 * 1024 * 1024  # elements in the full problem
PER_CORE = TOTAL // N_CORES  # 4,194,304
FREE = PER_CORE // P  # 32,768 f32 per partition
F = 2048  # tile free-dim
NT = FREE // F  # 16 tiles per core
MMN = 512  # matmul moving free-dim (one PSUM bank; ISA max)
NCH = F // MMN  # matmul chunks per tile
SMOOTH = 1e-05

_nc_cache = None


def _build_bass():
    nc = bacc.Bacc(None, target_bir_lowering=False)
    x = nc.dram_tensor("input", [P, FREE], mybir.dt.float32, kind="ExternalInput")
    t = nc.dram_tensor("target", [P, FREE], mybir.dt.int32, kind="ExternalInput")
    o_sums = nc.dram_tensor(
        "o_sums", [1, 6 * MMN], mybir.dt.float32, kind="ExternalOutput"
    )
    o_st = nc.dram_tensor("o_st", [P, NT], mybir.dt.float32, kind="ExternalOutput")

    with TileContext(nc) as tc:
        with (
            tc.tile_pool(name="work", bufs=2) as pool,
            tc.tile_pool(name="stats", bufs=1) as spool,
            tc.tile_pool(name="ps", bufs=1, space="PSUM") as psum,
        ):
            s_t = spool.tile([P, NT], mybir.dt.float32)
            ones = spool.tile([P, P], mybir.dt.bfloat16, tag="ones")
            ps_pt_a = psum.tile([P, MMN], mybir.dt.float32, tag="ps_pt_a")
            ps_pt_b = psum.tile([P, MMN], mybir.dt.float32, tag="ps_pt_b")
            ps_ppt_a = psum.tile([P, MMN], mybir.dt.float32, tag="ps_ppt_a")
            ps_ppt_b = psum.tile([P, MMN], mybir.dt.float32, tag="ps_ppt_b")
            ps_pp_a = psum.tile([P, MMN], mybir.dt.float32, tag="ps_pp_a")
            ps_pp_b = psum.tile([P, MMN], mybir.dt.float32, tag="ps_pp_b")

            # emit the first tile's loads before the ones-memset so the
            # sync queue reaches them as early as possible
            xts, tts = [], []
            for i in range(NT):
                xt = pool.tile([P, F], mybir.dt.float32, tag="xt", bufs=6)
                tt = pool.tile([P, F], mybir.dt.int32, tag="tt", bufs=6)
                nc.sync.dma_start(xt[:], x[:, i * F : (i + 1) * F])
                nc.sync.dma_start(tt[:], t[:, i * F : (i + 1) * F])
                xts.append(xt)
                tts.append(tt)
                if i == 0:
                    nc.any.memset(ones, 1.0)

            for i in range(NT):
                xt, tt = xts[i], tts[i]
                ptb = pool.tile([P, 2 * F], mybir.dt.bfloat16, tag="ptb", bufs=3)
                uwsq = pool.tile([P, 3 * F], mybir.dt.bfloat16, tag="uwsq")
                p_, tb = ptb[:, :F], ptb[:, F:]
                u, w, sq = uwsq[:, :F], uwsq[:, F : 2 * F], uwsq[:, 2 * F :]

                # p = sigmoid(x); tb = float(t) with S_t row-accum   [ACT]
                nc.scalar.activation(
                    p_, xt[:], mybir.ActivationFunctionType.Sigmoid
                )
                nc.scalar.activation(
                    tb,
                    tt[:],
                    mybir.ActivationFunctionType.Copy,
                    accum_out=s_t[:, i : i + 1],
                )

                chains = (
                    (0, (ps_pt_a, ps_pt_b)),  # u chunks
                    (F, (ps_ppt_a, ps_ppt_b)),  # w chunks
                    (2 * F, (ps_pp_a, ps_pp_b)),  # sq chunks
                )

                def mms(j0, nch):
                    # column-sum accumulation chains; each chain
                    # alternates two PSUM banks to pipeline the RMW  [PE]
                    for base, banks in chains:
                        for jj in range(nch):
                            j = j0 + jj
                            nc.tensor.matmul(
                                banks[j % 2][:],
                                ones[:],
                                uwsq[:, base + j * MMN : base + (j + 1) * MMN],
                                start=(i == 0 and j < 2),
                                stop=(i == NT - 1 and j >= NCH - 2),
                            )

                if i < NT - 1:
                    # u = p·t, w = u·p = p²t, sq = p² (bf16 2x mode) [DVE]
                    nc.vector.tensor_tensor(
                        out=u, in0=p_, in1=tb, op=mybir.AluOpType.mult
                    )
                    nc.vector.tensor_tensor(
                        out=w, in0=u, in1=p_, op=mybir.AluOpType.mult
                    )
                    nc.vector.tensor_tensor(
                        out=sq, in0=p_, in1=p_, op=mybir.AluOpType.mult
                    )
                    mms(0, NCH)
                else:
                    # last tile: u = p·t from the int32 tile (1x) so u
                    # doesn't wait on the S_t copy, and 512-col sub-slices
                    # so the PE chains drain as chunks are produced
                    for c in range(NCH):
                        sl = slice(c * MMN, (c + 1) * MMN)
                        nc.vector.tensor_tensor(
                            out=u[:, sl], in0=p_[:, sl], in1=tt[:, sl],
                            op=mybir.AluOpType.mult,
                        )
                        nc.vector.tensor_tensor(
                            out=w[:, sl], in0=u[:, sl], in1=p_[:, sl],
                            op=mybir.AluOpType.mult,
                        )
                        nc.vector.tensor_tensor(
                            out=sq[:, sl], in0=p_[:, sl], in1=p_[:, sl],
                            op=mybir.AluOpType.mult,
                        )
                        mms(c, 1)

            fin = spool.tile([1, 6 * MMN], mybir.dt.float32, tag="fin")
            for k, ps in enumerate(
                (ps_pt_a, ps_pt_b, ps_ppt_a, ps_ppt_b, ps_pp_a, ps_pp_b)
            ):
                dst = fin[:, k * MMN : (k + 1) * MMN]
                if k % 2 == 0:
                    nc.vector.tensor_copy(dst, ps[0:1, :])
                else:
                    nc.scalar.copy(dst, ps[0:1, :])
                if k == 3:
                    # fire the S_pt/S_ppt half while the S_pp copies run so
                    # the final (teardown-gating) DMA is only the small rest
                    nc.sync.dma_start(o_sums[:, : 4 * MMN], fin[:, : 4 * MMN])
            nc.sync.dma_start(o_sums[:, 4 * MMN :], fin[:, 4 * MMN :])
            nc.sync.dma_start(o_st[:], s_t[:])
    nc.finalize()
    return nc


def _get_nc():
    global _nc_cache
    if _nc_cache is None:
        _nc_cache = _build_bass()
    return _nc_cache


def kernel(input, target, _trace=False):
    x = np.ascontiguousarray(np.asarray(input, dtype=np.float32)).reshape(
        N_CORES, P, FREE
    )
    t = np.ascontiguousarray(np.asarray(target, dtype=np.int32)).reshape(
        N_CORES, P, FREE
    )
    in_maps = [{"input": x[i], "target": t[i]} for i in range(N_CORES)]

    nc = _get_nc()
    res = run_bass_kernel_spmd(
        nc, in_maps, core_ids=list(range(N_CORES)), trace=_trace
    )
    kernel.last_results = res

    s_pt = s_ppt = s_pp = s_t = 0.0
    for r in res.results:
        sums = r["o_sums"].astype(np.float64)
        s_pt += float(sums[0, 0 : 2 * MMN].sum())
        s_ppt += float(sums[0, 2 * MMN : 4 * MMN].sum())
        s_pp += float(sums[0, 4 * MMN :].sum())
        s_t += float(r["o_st"].astype(np.float64).sum())

    c1 = float(s_t)
    c0 = float(TOTAL - s_t)
    w0 = 1.0 / (c0 + SMOOTH) ** 2
    w1 = 1.0 / (c1 + SMOOTH) ** 2
    intersection = w1 * s_pt
    denominator = w0 * (s_pp - s_ppt) + w1 * (s_ppt + c1)
    dice = 1.0 - (2.0 * intersection + SMOOTH) / (denominator + SMOOTH)
    return np.asarray(dice, dtype=np.float32)



# revision 4
# speedup vs baseline: 1.1841x; 1.1841x over previous
"""Balanced dice loss (histogram binning) on 8 Trainium2 NeuronCores.

Math: with t ∈ {0,1} and p = sigmoid(x), the loss needs four global sums:
    S_t   = Σ t            (the bincount)
    S_pt  = Σ p·t
    S_pp  = Σ p²
    S_ppt = Σ p²·t
Then with c1 = S_t, c0 = N − c1, w0 = 1/(c0+s)², w1 = 1/(c1+s)²:
    intersection = w1·S_pt
    denominator  = w0·(S_pp − S_ppt) + w1·(S_ppt + c1)
    dice = 1 − (2·I + s)/(D + s)

Device kernel (data-parallel over 8 cores, batch-sharded). The target is
cast to int16 on host (values are {0,1}), cutting HBM traffic from 32MB
to 24MB per core. Per [128,4096] tile:
    ACT : p = sigmoid(x) (bf16) — the only ACT pass
    DVE : u = p·t (bf16×int16, 2x mode), w = u·p, sq = p·p (all 2x)
    PE  : four ones[128,128]-stationary column-sum chains over 512-col
          chunks: Σu → S_pt, Σw → S_ppt, Σsq → S_pp, and Σ over the int16
          t tile bitcast to fp16 (value 1 = denormal 2^-24, summed exactly
          in f32 PSUM) → S_t·2^-24. Each chain alternates two PSUM banks
          (8 banks total) and accumulates across ALL tiles; one eviction
          at the end.
The first tile's ops are emitted in 2048-col halves (compute starts as
soon as half the tile has landed) and the last tile's in 1024-col slices
(short drain after the final DMA). Host sums partials in float64.
"""

import numpy as np

import concourse.bacc as bacc
import concourse.mybir as mybir
from concourse.bass_utils import run_bass_kernel_spmd
from concourse.tile import TileContext

N_CORES = 8
P = 128
TOTAL = 32 * 1024 * 1024  # elements in the full problem
PER_CORE = TOTAL // N_CORES  # 4,194,304
FREE = PER_CORE // P  # 32,768 per partition
F = 4096  # tile free-dim
NT = FREE // F  # 8 tiles per core
MMN = 512  # matmul moving free-dim (one PSUM bank; ISA max)
SMOOTH = 1e-05

PS_NAMES = ("ua", "ub", "wa", "wb", "qa", "qb", "ta", "tb")

_nc_cache = None


def _slices(i):
    # sub-instruction column slices within a tile
    if i == 0:
        return [(0, 2048), (2048, 4096)]
    if i == NT - 1:
        return [(0, 1024), (1024, 2048), (2048, 3072), (3072, 4096)]
    return [(0, F)]


def _build_bass():
    nc = bacc.Bacc(None, target_bir_lowering=False)
    x = nc.dram_tensor("input", [P, FREE], mybir.dt.float32, kind="ExternalInput")
    t = nc.dram_tensor("target", [P, FREE], mybir.dt.int16, kind="ExternalInput")
    # 8 chain partials, one [1, MMN] row each, order PS_NAMES
    o_fin = nc.dram_tensor(
        "o_fin", [1, 8 * MMN], mybir.dt.float32, kind="ExternalOutput"
    )

    with TileContext(nc) as tc:
        with (
            tc.tile_pool(name="work", bufs=2) as pool,
            tc.tile_pool(name="stats", bufs=1) as spool,
            tc.tile_pool(name="ps", bufs=1, space="PSUM") as psum,
        ):
            ones = spool.tile([P, P], mybir.dt.bfloat16, tag="ones")
            ps = {
                name: psum.tile(
                    [P, MMN], mybir.dt.float32, tag=f"ps_{name}", name=f"ps_{name}"
                )
                for name in PS_NAMES
            }
            ps_first = {name: True for name in PS_NAMES}

            # emit all loads first so the sync queue streams them; memset
            # rides after the first tile's first-half loads
            xts, tts = [], []
            for i in range(NT):
                xt = pool.tile([P, F], mybir.dt.float32, tag="xt", bufs=4)
                tt = pool.tile([P, F], mybir.dt.int16, tag="tt", bufs=4)
                for k, (a, b) in enumerate(_slices(i)):
                    nc.sync.dma_start(xt[:, a:b], x[:, i * F + a : i * F + b])
                    nc.sync.dma_start(tt[:, a:b], t[:, i * F + a : i * F + b])
                    if i == 0 and k == 0:
                        nc.any.memset(ones, 1.0)
                xts.append(xt)
                tts.append(tt)

            total_chunks = FREE // MMN  # per chain, across the whole kernel
            emitted = {"u": 0, "w": 0, "q": 0, "t": 0}

            def chain(stream, pair, moving):
                emitted[stream] += 1
                name = pair[emitted[stream] % 2]  # alternate banks
                nc.tensor.matmul(
                    ps[name][:],
                    ones[:],
                    moving,
                    start=ps_first[name],
                    stop=emitted[stream] > total_chunks - 2,
                )
                ps_first[name] = False

            for i in range(NT):
                xt, tt = xts[i], tts[i]
                p_ = pool.tile([P, F], mybir.dt.bfloat16, tag="p", bufs=3)
                uwq = pool.tile([P, 3 * F], mybir.dt.bfloat16, tag="uwq", bufs=2)
                t16 = tt[:].bitcast(mybir.dt.float16)

                for a, b in _slices(i):
                    u = uwq[:, a:b]
                    w = uwq[:, F + a : F + b]
                    sq = uwq[:, 2 * F + a : 2 * F + b]
                    pv = p_[:, a:b]

                    # S_t chain first: only needs the t DMA  [PE]
                    for j in range(a // MMN, b // MMN):
                        chain("t", ("ta", "tb"), t16[:, j * MMN : (j + 1) * MMN])

                    # p = sigmoid(x)  [ACT]
                    nc.scalar.activation(
                        pv, xt[:, a:b], mybir.ActivationFunctionType.Sigmoid
                    )
                    # products (all 2-byte operands -> DVE 2x mode)  [DVE]
                    nc.vector.tensor_tensor(
                        out=u, in0=pv, in1=tt[:, a:b], op=mybir.AluOpType.mult
                    )
                    nc.vector.tensor_tensor(
                        out=w, in0=u, in1=pv, op=mybir.AluOpType.mult
                    )
                    nc.vector.tensor_tensor(
                        out=sq, in0=pv, in1=pv, op=mybir.AluOpType.mult
                    )
                    # column-sum chains  [PE]
                    for j in range(a // MMN, b // MMN):
                        chain("u", ("ua", "ub"), uwq[:, j * MMN : (j + 1) * MMN])
                    for j in range(a // MMN, b // MMN):
                        chain(
                            "w", ("wa", "wb"),
                            uwq[:, F + j * MMN : F + (j + 1) * MMN],
                        )
                    for j in range(a // MMN, b // MMN):
                        chain(
                            "q", ("qa", "qb"),
                            uwq[:, 2 * F + j * MMN : 2 * F + (j + 1) * MMN],
                        )

            # evict the 8 psum partial rows; split between DVE and ACT
            fin = spool.tile([1, 8 * MMN], mybir.dt.float32, tag="fin")
            for k, name in enumerate(PS_NAMES):
                dst = fin[:, k * MMN : (k + 1) * MMN]
                if k % 2 == 0:
                    nc.vector.tensor_copy(dst, ps[name][0:1, :])
                else:
                    nc.scalar.copy(dst, ps[name][0:1, :])
                if k == 5:
                    nc.sync.dma_start(o_fin[:, : 6 * MMN], fin[:, : 6 * MMN])
            nc.sync.dma_start(o_fin[:, 6 * MMN :], fin[:, 6 * MMN :])
    nc.finalize()
    return nc


def _get_nc():
    global _nc_cache
    if _nc_cache is None:
        _nc_cache = _build_bass()
    return _nc_cache


def kernel(input, target, _trace=False):
    x = np.ascontiguousarray(np.asarray(input, dtype=np.float32)).reshape(
        N_CORES, P, FREE
    )
    t = (
        np.ascontiguousarray(np.asarray(target))
        .reshape(N_CORES, P, FREE)
        .astype(np.int16)
    )
    in_maps = [{"input": x[i], "target": t[i]} for i in range(N_CORES)]

    nc = _get_nc()
    res = run_bass_kernel_spmd(
        nc, in_maps, core_ids=list(range(N_CORES)), trace=_trace
    )
    kernel.last_results = res

    s_pt = s_ppt = s_pp = s_t = 0.0
    for r in res.results:
        f = r["o_fin"].astype(np.float64).reshape(8, MMN)
        s_pt += float(f[0].sum() + f[1].sum())
        s_ppt += float(f[2].sum() + f[3].sum())
        s_pp += float(f[4].sum() + f[5].sum())
        s_t += float(f[6].sum() + f[7].sum()) * (2.0**24)

    c1 = float(s_t)
    c0 = float(TOTAL - s_t)
    w0 = 1.0 / (c0 + SMOOTH) ** 2
    w1 = 1.0 / (c1 + SMOOTH) ** 2
    intersection = w1 * s_pt
    denominator = w0 * (s_pp - s_ppt) + w1 * (s_ppt + c1)
    dice = 1.0 - (2.0 * intersection + SMOOTH) / (denominator + SMOOTH)
    return np.asarray(dice, dtype=np.float32)


# revision 7
# speedup vs baseline: 1.2655x; 1.0687x over previous
"""Balanced dice loss (histogram binning) on 8 Trainium2 NeuronCores.

Math: with t ∈ {0,1} and p = sigmoid(x), the loss needs four global sums:
    S_t   = Σ t            (the bincount)
    S_pt  = Σ p·t
    S_pp  = Σ p²
    S_ppt = Σ p²·t
Then with c1 = S_t, c0 = N − c1, w0 = 1/(c0+s)², w1 = 1/(c1+s)²:
    intersection = w1·S_pt
    denominator  = w0·(S_pp − S_ppt) + w1·(S_ppt + c1)
    dice = 1 − (2·I + s)/(D + s)

Device kernel (data-parallel over 8 cores, batch-sharded). Host casts
x to bf16 and target to int16 (values are {0,1}): HBM traffic drops from
32MB to 16MB per core. Per [128,4096] tile:
    ACT : p = sigmoid(x) (bf16) + for tiles 1..6 a copy pass over the
          int16 t with accum_out → S_t partials (ACT has slack; this
          keeps the PE under the DMA roofline)
    DVE : u = p·t (bf16×int16, 2x mode), w = u·p, sq = p·p (all 2x)
    PE  : ones[128,128]-stationary column-sum chains over 512-col chunks:
          Σu → S_pt, Σw → S_ppt, Σsq → S_pp, plus for tiles {0,7} a chain
          over the int16 t bitcast to fp16 (value 1 = denormal 2^-24,
          summed exactly in f32 PSUM) → S_t·2^-24. Each chain alternates
          two PSUM banks and accumulates across all tiles; one eviction
          at the end.
The first tile is emitted in 512/1536/2048-col slices (PE's t-chain
starts as soon as the first 128KB lands), the last tile in 1024-col
slices (short drain after the final DMA). Host sums in float64.
"""

import numpy as np
import ml_dtypes

import concourse.bacc as bacc
import concourse.mybir as mybir
from concourse.bass_utils import run_bass_kernel_spmd
from concourse.tile import TileContext

N_CORES = 8
P = 128
TOTAL = 32 * 1024 * 1024  # elements in the full problem
PER_CORE = TOTAL // N_CORES  # 4,194,304
FREE = PER_CORE // P  # 32,768 per partition
F = 4096  # tile free-dim
NT = FREE // F  # 8 tiles per core
MMN = 512  # matmul moving free-dim (one PSUM bank; ISA max)
SMOOTH = 1e-05

PS_NAMES = ("ua", "ub", "wa", "wb", "qa", "qb", "ta", "tb")
PE_T_TILES = (0, NT - 1)  # tiles whose S_t goes through the PE denorm chain

_nc_cache = None


def _slices(i):
    # sub-instruction column slices within a tile
    if i == 0:
        return [(0, 512), (512, 2048), (2048, 4096)]
    if i == NT - 1:
        return [(0, 1024), (1024, 2048), (2048, 3072), (3072, 4096)]
    return [(0, F)]


def _build_bass():
    nc = bacc.Bacc(None, target_bir_lowering=False)
    x = nc.dram_tensor("input", [P, FREE], mybir.dt.bfloat16, kind="ExternalInput")
    t = nc.dram_tensor("target", [P, FREE], mybir.dt.int16, kind="ExternalInput")
    # 8 chain partials ([1, MMN] each, order PS_NAMES) then S_t ACT partials
    o_fin = nc.dram_tensor(
        "o_fin", [1, 8 * MMN], mybir.dt.float32, kind="ExternalOutput"
    )

    with TileContext(nc) as tc:
        with (
            tc.tile_pool(name="work", bufs=2) as pool,
            tc.tile_pool(name="stats", bufs=1) as spool,
            tc.tile_pool(name="ps", bufs=1, space="PSUM") as psum,
        ):
            ones = spool.tile([P, P], mybir.dt.bfloat16, tag="ones")
            nc.any.memset(ones, 1.0)
            s_t_acc = spool.tile([P, NT], mybir.dt.float32, tag="s_t_acc")
            nc.vector.memset(s_t_acc, 0.0)

            ps = {
                name: psum.tile(
                    [P, MMN], mybir.dt.float32, tag=f"ps_{name}", name=f"ps_{name}"
                )
                for name in PS_NAMES
            }
            ps_first = {name: True for name in PS_NAMES}

            # emit all loads first so the sync queue streams them; t before
            # x within each tile (the PE t-chain and DVE need t first)
            xts, tts = [], []
            for i in range(NT):
                xt = pool.tile([P, F], mybir.dt.bfloat16, tag="xt", bufs=5)
                tt = pool.tile([P, F], mybir.dt.int16, tag="tt", bufs=5)
                for a, b in _slices(i):
                    nc.sync.dma_start(tt[:, a:b], t[:, i * F + a : i * F + b])
                    nc.sync.dma_start(xt[:, a:b], x[:, i * F + a : i * F + b])
                xts.append(xt)
                tts.append(tt)

            # chunks per chain across the whole kernel (for stop flags)
            totals = {"u": FREE // MMN, "w": FREE // MMN, "q": FREE // MMN,
                      "t": len(PE_T_TILES) * F // MMN}
            emitted = {k: 0 for k in totals}

            def chain(stream, pair, moving):
                emitted[stream] += 1
                name = pair[emitted[stream] % 2]  # alternate banks
                nc.tensor.matmul(
                    ps[name][:],
                    ones[:],
                    moving,
                    start=ps_first[name],
                    stop=emitted[stream] > totals[stream] - 2,
                )
                ps_first[name] = False

            t_scr = spool.tile([P, F], mybir.dt.bfloat16, tag="t_scr")

            for i in range(NT):
                xt, tt = xts[i], tts[i]
                p_ = pool.tile([P, F], mybir.dt.bfloat16, tag="p", bufs=3)
                uwq = pool.tile([P, 3 * F], mybir.dt.bfloat16, tag="uwq", bufs=2)
                t16 = tt[:].bitcast(mybir.dt.float16)

                for a, b in _slices(i):
                    u = uwq[:, a:b]
                    w = uwq[:, F + a : F + b]
                    sq = uwq[:, 2 * F + a : 2 * F + b]
                    pv = p_[:, a:b]

                    if i in PE_T_TILES:
                        # S_t chain first: only needs the t DMA  [PE]
                        for j in range(a // MMN, b // MMN):
                            chain("t", ("ta", "tb"), t16[:, j * MMN : (j + 1) * MMN])

                    # p = sigmoid(x)  [ACT]
                    nc.scalar.activation(
                        pv, xt[:, a:b], mybir.ActivationFunctionType.Sigmoid
                    )
                    # products (all 2-byte operands -> DVE 2x mode)  [DVE]
                    nc.vector.tensor_tensor(
                        out=u, in0=pv, in1=tt[:, a:b], op=mybir.AluOpType.mult
                    )
                    nc.vector.tensor_tensor(
                        out=w, in0=u, in1=pv, op=mybir.AluOpType.mult
                    )
                    nc.vector.tensor_tensor(
                        out=sq, in0=pv, in1=pv, op=mybir.AluOpType.mult
                    )
                    # column-sum chains  [PE]
                    for j in range(a // MMN, b // MMN):
                        chain("u", ("ua", "ub"), uwq[:, j * MMN : (j + 1) * MMN])
                    for j in range(a // MMN, b // MMN):
                        chain(
                            "w", ("wa", "wb"),
                            uwq[:, F + j * MMN : F + (j + 1) * MMN],
                        )
                    for j in range(a // MMN, b // MMN):
                        chain(
                            "q", ("qa", "qb"),
                            uwq[:, 2 * F + j * MMN : 2 * F + (j + 1) * MMN],
                        )

                if i not in PE_T_TILES:
                    # S_t via copy-with-accum; output is a dead scratch  [ACT]
                    nc.scalar.activation(
                        t_scr[:],
                        tt[:],
                        mybir.ActivationFunctionType.Copy,
                        accum_out=s_t_acc[:, i : i + 1],
                    )

            # the ACT S_t partials [P, NT] are tiny; sum them on host
            o_st = nc.dram_tensor(
                "o_st", [P, NT], mybir.dt.float32, kind="ExternalOutput"
            )
            nc.sync.dma_start(o_st[:], s_t_acc[:])

            # evict the 8 psum partial rows; split between DVE and ACT
            fin = spool.tile([1, 8 * MMN], mybir.dt.float32, tag="fin")
            for k, name in enumerate(PS_NAMES):
                dst = fin[:, k * MMN : (k + 1) * MMN]
                if k % 2 == 0:
                    nc.vector.tensor_copy(dst, ps[name][0:1, :])
                else:
                    nc.scalar.copy(dst, ps[name][0:1, :])
                if k == 5:
                    nc.sync.dma_start(o_fin[:, : 6 * MMN], fin[:, : 6 * MMN])
            nc.sync.dma_start(o_fin[:, 6 * MMN :], fin[:, 6 * MMN :])
    nc.finalize()
    return nc


def _get_nc():
    global _nc_cache
    if _nc_cache is None:
        _nc_cache = _build_bass()
    return _nc_cache


def kernel(input, target, _trace=False):
    x = (
        np.ascontiguousarray(np.asarray(input, dtype=np.float32))
        .reshape(N_CORES, P, FREE)
        .astype(ml_dtypes.bfloat16)
    )
    t = (
        np.ascontiguousarray(np.asarray(target))
        .reshape(N_CORES, P, FREE)
        .astype(np.int16)
    )
    in_maps = [{"input": x[i], "target": t[i]} for i in range(N_CORES)]

    nc = _get_nc()
    res = run_bass_kernel_spmd(
        nc, in_maps, core_ids=list(range(N_CORES)), trace=_trace
    )
    kernel.last_results = res

    s_pt = s_ppt = s_pp = s_t = 0.0
    for r in res.results:
        f = r["o_fin"].astype(np.float64)[0]
        s_pt += float(f[0 : 2 * MMN].sum())
        s_ppt += float(f[2 * MMN : 4 * MMN].sum())
        s_pp += float(f[4 * MMN : 6 * MMN].sum())
        s_t += float(f[6 * MMN : 8 * MMN].sum()) * (2.0**24)
        s_t += float(r["o_st"].astype(np.float64).sum())

    c1 = float(s_t)
    c0 = float(TOTAL - s_t)
    w0 = 1.0 / (c0 + SMOOTH) ** 2
    w1 = 1.0 / (c1 + SMOOTH) ** 2
    intersection = w1 * s_pt
    denominator = w0 * (s_pp - s_ppt) + w1 * (s_ppt + c1)
    dice = 1.0 - (2.0 * intersection + SMOOTH) / (denominator + SMOOTH)
    return np.asarray(dice, dtype=np.float32)


# revision 8
# speedup vs baseline: 1.2809x; 1.0122x over previous
"""Balanced dice loss (histogram binning) on 8 Trainium2 NeuronCores.

Math: with t ∈ {0,1} and p = sigmoid(x), the loss needs four global sums:
    S_t   = Σ t            (the bincount)
    S_pt  = Σ p·t
    S_pp  = Σ p²
    S_ppt = Σ p²·t
Then with c1 = S_t, c0 = N − c1, w0 = 1/(c0+s)², w1 = 1/(c1+s)²:
    intersection = w1·S_pt
    denominator  = w0·(S_pp − S_ppt) + w1·(S_ppt + c1)
    dice = 1 − (2·I + s)/(D + s)

Device kernel (data-parallel over 8 cores, batch-sharded). Host casts
x to bf16 and target to int16 (values are {0,1}): HBM traffic drops from
32MB to 16MB per core. Per [128,4096] tile:
    ACT : p = sigmoid(x) (bf16) + for tiles 1..6 a copy pass over the
          int16 t with accum_out → S_t partials (ACT has slack; this
          keeps the PE under the DMA roofline)
    DVE : u = p·t (bf16×int16, 2x mode), w = u·p, sq = p·p (all 2x)
    PE  : ones[128,128]-stationary column-sum chains over 512-col chunks:
          Σu → S_pt, Σw → S_ppt, Σsq → S_pp, plus for tiles {0,7} a chain
          over the int16 t bitcast to fp16 (value 1 = denormal 2^-24,
          summed exactly in f32 PSUM) → S_t·2^-24. Each chain alternates
          two PSUM banks and accumulates across all tiles; one eviction
          at the end.
The first tile is emitted in 512/1536/2048-col slices (PE's t-chain
starts as soon as the first 128KB lands), the last tile in 1024-col
slices (short drain after the final DMA). Host sums in float64.
"""

import numpy as np
import ml_dtypes

import concourse.bacc as bacc
import concourse.mybir as mybir
from concourse.bass_utils import run_bass_kernel_spmd
from concourse.tile import TileContext

N_CORES = 8
P = 128
TOTAL = 32 * 1024 * 1024  # elements in the full problem
PER_CORE = TOTAL // N_CORES  # 4,194,304
FREE = PER_CORE // P  # 32,768 per partition
F = 4096  # tile free-dim
NT = FREE // F  # 8 tiles per core
MMN = 512  # matmul moving free-dim (one PSUM bank; ISA max)
SMOOTH = 1e-05

PS_NAMES = ("ua", "ub", "wa", "wb", "qa", "qb", "ta", "tb")
PE_T_TILES = (0, NT - 1)  # tiles whose S_t goes through the PE denorm chain

_nc_cache = None


def _slices(i):
    # sub-instruction column slices within a tile
    if i == 0:
        return [(0, 512), (512, 2048), (2048, 4096)]
    if i == NT - 1:
        return [(0, 1024), (1024, 2048), (2048, 3072), (3072, 4096)]
    return [(0, F)]


def _build_bass():
    nc = bacc.Bacc(None, target_bir_lowering=False)
    x = nc.dram_tensor("input", [P, FREE], mybir.dt.bfloat16, kind="ExternalInput")
    t = nc.dram_tensor("target", [P, FREE], mybir.dt.int16, kind="ExternalInput")
    # 8 chain partials ([1, MMN] each, order PS_NAMES) then S_t ACT partials
    o_fin = nc.dram_tensor(
        "o_fin", [1, 8 * MMN], mybir.dt.float32, kind="ExternalOutput"
    )

    with TileContext(nc) as tc:
        with (
            tc.tile_pool(name="work", bufs=2) as pool,
            tc.tile_pool(name="stats", bufs=1) as spool,
            tc.tile_pool(name="ps", bufs=1, space="PSUM") as psum,
        ):
            ones = spool.tile([P, P], mybir.dt.bfloat16, tag="ones")
            nc.any.memset(ones, 1.0)
            s_t_acc = spool.tile([P, NT], mybir.dt.float32, tag="s_t_acc")
            nc.vector.memset(s_t_acc, 0.0)

            ps = {
                name: psum.tile(
                    [P, MMN], mybir.dt.float32, tag=f"ps_{name}", name=f"ps_{name}"
                )
                for name in PS_NAMES
            }
            ps_first = {name: True for name in PS_NAMES}

            # emit all loads first so the sync queue streams them; t before
            # x within each tile (the PE t-chain and DVE need t first)
            xts, tts = [], []
            for i in range(NT):
                xt = pool.tile([P, F], mybir.dt.bfloat16, tag="xt", bufs=4)
                tt = pool.tile([P, F], mybir.dt.int16, tag="tt", bufs=4)
                for a, b in _slices(i):
                    nc.sync.dma_start(tt[:, a:b], t[:, i * F + a : i * F + b])
                    nc.sync.dma_start(xt[:, a:b], x[:, i * F + a : i * F + b])
                xts.append(xt)
                tts.append(tt)

            # chunks per chain across the whole kernel (for stop flags)
            totals = {"u": FREE // MMN, "w": FREE // MMN, "q": FREE // MMN,
                      "t": len(PE_T_TILES) * F // MMN}
            emitted = {k: 0 for k in totals}

            def chain(stream, pair, moving):
                emitted[stream] += 1
                name = pair[emitted[stream] % 2]  # alternate banks
                nc.tensor.matmul(
                    ps[name][:],
                    ones[:],
                    moving,
                    start=ps_first[name],
                    stop=emitted[stream] > totals[stream] - 2,
                )
                ps_first[name] = False

            t_scr = spool.tile([P, F], mybir.dt.bfloat16, tag="t_scr")

            for i in range(NT):
                xt, tt = xts[i], tts[i]
                p_ = pool.tile([P, F], mybir.dt.bfloat16, tag="p", bufs=4)
                uwq = pool.tile([P, 3 * F], mybir.dt.bfloat16, tag="uwq", bufs=3)
                t16 = tt[:].bitcast(mybir.dt.float16)

                for a, b in _slices(i):
                    u = uwq[:, a:b]
                    w = uwq[:, F + a : F + b]
                    sq = uwq[:, 2 * F + a : 2 * F + b]
                    pv = p_[:, a:b]

                    if i in PE_T_TILES:
                        # S_t chain first: only needs the t DMA  [PE]
                        for j in range(a // MMN, b // MMN):
                            chain("t", ("ta", "tb"), t16[:, j * MMN : (j + 1) * MMN])

                    # p = sigmoid(x)  [ACT]
                    nc.scalar.activation(
                        pv, xt[:, a:b], mybir.ActivationFunctionType.Sigmoid
                    )
                    # products (all 2-byte operands -> DVE 2x mode)  [DVE]
                    nc.vector.tensor_tensor(
                        out=u, in0=pv, in1=tt[:, a:b], op=mybir.AluOpType.mult
                    )
                    nc.vector.tensor_tensor(
                        out=w, in0=u, in1=pv, op=mybir.AluOpType.mult
                    )
                    nc.vector.tensor_tensor(
                        out=sq, in0=pv, in1=pv, op=mybir.AluOpType.mult
                    )
                    # column-sum chains  [PE]
                    for j in range(a // MMN, b // MMN):
                        chain("u", ("ua", "ub"), uwq[:, j * MMN : (j + 1) * MMN])
                    for j in range(a // MMN, b // MMN):
                        chain(
                            "w", ("wa", "wb"),
                            uwq[:, F + j * MMN : F + (j + 1) * MMN],
                        )
                    for j in range(a // MMN, b // MMN):
                        chain(
                            "q", ("qa", "qb"),
                            uwq[:, 2 * F + j * MMN : 2 * F + (j + 1) * MMN],
                        )

                # S_t via copy-with-accum for the PREVIOUS tile (delayed
                # one tile so sigmoid(i+1) isn't stuck behind copy(i) on
                # the ACT queue); output is a dead scratch  [ACT]
                for j in (i - 1, i) if i == NT - 1 else (i - 1,):
                    if 0 <= j and j not in PE_T_TILES:
                        nc.scalar.activation(
                            t_scr[:],
                            tts[j][:],
                            mybir.ActivationFunctionType.Copy,
                            accum_out=s_t_acc[:, j : j + 1],
                        )

            # the ACT S_t partials [P, NT] are tiny; sum them on host
            o_st = nc.dram_tensor(
                "o_st", [P, NT], mybir.dt.float32, kind="ExternalOutput"
            )
            nc.sync.dma_start(o_st[:], s_t_acc[:])

            # evict the 8 psum partial rows; split between DVE and ACT
            fin = spool.tile([1, 8 * MMN], mybir.dt.float32, tag="fin")
            for k, name in enumerate(PS_NAMES):
                dst = fin[:, k * MMN : (k + 1) * MMN]
                if k % 2 == 0:
                    nc.vector.tensor_copy(dst, ps[name][0:1, :])
                else:
                    nc.scalar.copy(dst, ps[name][0:1, :])
                if k == 5:
                    nc.sync.dma_start(o_fin[:, : 6 * MMN], fin[:, : 6 * MMN])
            nc.sync.dma_start(o_fin[:, 6 * MMN :], fin[:, 6 * MMN :])
    nc.finalize()
    return nc


def _get_nc():
    global _nc_cache
    if _nc_cache is None:
        _nc_cache = _build_bass()
    return _nc_cache


def kernel(input, target, _trace=False):
    x = (
        np.ascontiguousarray(np.asarray(input, dtype=np.float32))
        .reshape(N_CORES, P, FREE)
        .astype(ml_dtypes.bfloat16)
    )
    t = (
        np.ascontiguousarray(np.asarray(target))
        .reshape(N_CORES, P, FREE)
        .astype(np.int16)
    )
    in_maps = [{"input": x[i], "target": t[i]} for i in range(N_CORES)]

    nc = _get_nc()
    res = run_bass_kernel_spmd(
        nc, in_maps, core_ids=list(range(N_CORES)), trace=_trace
    )
    kernel.last_results = res

    s_pt = s_ppt = s_pp = s_t = 0.0
    for r in res.results:
        f = r["o_fin"].astype(np.float64)[0]
        s_pt += float(f[0 : 2 * MMN].sum())
        s_ppt += float(f[2 * MMN : 4 * MMN].sum())
        s_pp += float(f[4 * MMN : 6 * MMN].sum())
        s_t += float(f[6 * MMN : 8 * MMN].sum()) * (2.0**24)
        s_t += float(r["o_st"].astype(np.float64).sum())

    c1 = float(s_t)
    c0 = float(TOTAL - s_t)
    w0 = 1.0 / (c0 + SMOOTH) ** 2
    w1 = 1.0 / (c1 + SMOOTH) ** 2
    intersection = w1 * s_pt
    denominator = w0 * (s_pp - s_ppt) + w1 * (s_ppt + c1)
    dice = 1.0 - (2.0 * intersection + SMOOTH) / (denominator + SMOOTH)
    return np.asarray(dice, dtype=np.float32)
